# revision 22
# baseline (speedup 1.0000x reference)
"""Trainium2 Bass kernel for nn_SSMLayer_17514876633683.

Math: the reference SSM state update broadcasts the input over H and starts
from zero state, so state[b,:,h] is identical for every h.  The whole layer
collapses to:
    z_t[b]    = A @ z_{t-1}[b] + B @ x[b,t]          (z in R^S, S=128)
    c[b,t]    = Cbar . z_t[b]                         (Cbar = C.mean(0))
    y_pre     = c[b,t] + (x @ D.T)[b,t,:]
    y         = LN(gelu(y_pre) + x) * gamma + beta

Sharding: 8 cores = 4 batches x 2 time-halves.  Every core runs the same
SPMD program: "scan all 512 steps of the provided x, output rows 256..511".
The first-half core of each batch receives x zero-padded at the front so its
output rows land in [256, 512) too.

Scan mapping on device (per core, its batch):
  U = B @ x^T                               (S x T)       - PE matmuls
  R_j = sum_r A^(Q-1-r) U[:, jQ+r]          (chunk summaries, Q=16, 32 chunks)
  Z_j = sum_{L<LZ} (A^Q)^L R_{j-1-L}        (chunk-boundary states; LZ lag
                                             matmuls; higher lags dropped when
                                             ||(A^Q)^L|| is negligible)
  c^T[j,i] = g_i . Z_j + sum_{k<i} g_{i-1-k} . U[:, jQ+k]  (g_k = (A^T)^k Cbar)
All A-power / g weight matrices are precomputed host-side from the inputs.
Matmul operands are bf16 (fp32 PSUM accumulation); the residual/layernorm
path stays fp32.  x is transposed on the (pre-warmed) tensor engine; all
weights arrive in one packed DMA per dtype.
"""

import sys
from contextlib import ExitStack

sys.path.insert(0, "/opt/trn_rl_repo")

import ml_dtypes
import numpy as np

import concourse.bass as bass  # noqa: F401
import concourse.mybir as mybir
import concourse.tile as tile
from concourse import bacc, bass_utils
from concourse.masks import make_identity

# Problem shapes (hardcoded per the harness contract).
BSZ, T, H, S = 4, 512, 512, 128
Q = 16           # scan chunk length
NCH = T // Q     # 32 chunks
TOUT = 256       # output rows per core
LN_EPS = 1e-5
NCORES = 8
NWARM = 17       # PE warmup matmuls (N=256, ~3.6us busy -> HAM un-throttle)
TRUNC_TOL = 1e-5

F32 = mybir.dt.float32
BF16 = mybir.dt.bfloat16
BF16_NP = ml_dtypes.bfloat16
AF = mybir.ActivationFunctionType
ALU = mybir.AluOpType

# PBT element offsets (bf16 pack: B^T | APOW | APQL | GW)
_O_BT = 0
_O_AP = _O_BT + 4 * S
_O_AQ = _O_AP + Q * S


def _host_weights(A, Bm, Cm):
    """Precompute scan weights; returns (APOW, APQL, GW, LZ) as float64."""
    A64 = A.astype(np.float64)
    Cbar = Cm.astype(np.float64).mean(axis=0)          # (S,)

    pows = [np.eye(S)]
    for _ in range(Q):
        pows.append(pows[-1] @ A64)                    # pows[k] = A^k
    A16 = pows[Q]

    # lhsT tiles for R: column block r holds (A^(Q-1-r))^T
    APOW = np.concatenate([pows[Q - 1 - r].T for r in range(Q)], axis=1)

    # boundary-lag powers, truncated once ||(A^Q)^L|| is negligible
    q16 = [np.eye(S)]
    while len(q16) < NCH - 1:
        nxt = q16[-1] @ A16
        if np.linalg.norm(nxt, 2) < TRUNC_TOL:
            break
        q16.append(nxt)
    LZ = len(q16)
    APQL = np.concatenate([m.T for m in q16], axis=1)

    g = [pows[k].T @ Cbar for k in range(Q)]           # g_k = (A^T)^k Cbar
    G16 = np.stack(g, axis=1)                          # (S, Q)
    WTRI = np.zeros((S, Q * Q))
    for k in range(Q):
        for i in range(Q):
            if i > k:
                WTRI[:, k * Q + i] = g[i - 1 - k]
    GW = np.concatenate([G16, WTRI], axis=1)           # (S, Q + Q*Q)

    return APOW, APQL, GW, LZ


def _emit(tc, aps, apply_gamma_beta, LZ):
    nc = tc.nc
    xb, pbt, pdt, p32, yout = (aps["xb"], aps["pbt"], aps["pdt"], aps["p32"],
                               aps["yout"])

    ctx = ExitStack()
    cpool = ctx.enter_context(tc.tile_pool(name="const", bufs=1))
    wpool = ctx.enter_context(tc.tile_pool(name="work", bufs=2))
    tpp = ctx.enter_context(tc.tile_pool(name="tpp", bufs=3, space="PSUM"))
    spp = ctx.enter_context(tc.tile_pool(name="spp", bufs=1, space="PSUM"))
    ypp = ctx.enter_context(tc.tile_pool(name="ypp", bufs=2, space="PSUM"))
    dpool = ctx.enter_context(tc.tile_pool(name="dram", bufs=1, space="DRAM"))

    # ---- PE warmup + gelu table preload (runs while input DMAs land) ------
    # One accumulation group so the matmuls pipeline back-to-back and trip
    # the HAM un-throttle (isolated matmuls never warm the clock gate).
    warm_sb = cpool.tile([128, 256], BF16, tag="warm_sb")
    nc.gpsimd.memset(warm_sb[:], 0.0)
    wp = tpp.tile([128, 256], F32, tag="tp", name="warm_ps")
    for i in range(NWARM):
        nc.tensor.matmul(wp[:], lhsT=warm_sb[:, :128], rhs=warm_sb[:],
                         start=(i == 0), stop=(i == NWARM - 1))
    gsc = cpool.tile([128, 1], F32, tag="gsc")
    nc.gpsimd.memset(gsc[:], 0.0)
    nc.scalar.activation(gsc[:], gsc[:], AF.Gelu)

    eps_sb = cpool.tile([128, 1], F32, tag="eps_sb")
    nc.gpsimd.memset(eps_sb[:], LN_EPS)
    big_sb = cpool.tile([128, 1], F32, tag="big_sb")
    nc.gpsimd.memset(big_sb[:], 1.0e4)

    # ---- identity built on device (no DMA on the transpose path) ----------
    id_sb = cpool.tile([128, 128], BF16, tag="id_sb")
    make_identity(nc, id_sb[:])

    # ---- input loads: ordered by need-time --------------------------------
    xb_sb = cpool.tile([128, 4, H], BF16, tag="xb_sb")
    nc.sync.dma_start(xb_sb[:], xb.rearrange("(tt p) h -> p tt h", p=128))
    PBT = cpool.tile([128, pbt.shape[1]], BF16, tag="PBT")
    nc.sync.dma_start(PBT[:], pbt)
    PDT = cpool.tile([128, 4, H], BF16, tag="PDT")
    nc.sync.dma_start(PDT[:], pdt.rearrange("p (hh o) -> p hh o", hh=4))
    P32 = cpool.tile([128, p32.shape[1]], F32, tag="P32")
    nc.gpsimd.dma_start(P32[:], p32)

    Bt_sb = PBT[:, _O_BT:_O_BT + 4 * S].rearrange("p (hh s) -> p hh s", hh=4)
    Dt_sb = PDT
    APOW_sb = PBT[:, _O_AP:_O_AP + Q * S]
    APQL_sb = PBT[:, _O_AQ:_O_AQ + LZ * S]
    GW_sb = PBT[:, _O_AQ + LZ * S:_O_AQ + LZ * S + Q + Q * Q]
    xres_sb = P32[:, 0:2 * H].rearrange("p (tt h) -> p tt h", tt=2)
    if apply_gamma_beta:
        gb_sb = P32[:, 2 * H:4 * H].rearrange("p (g h) -> p g h", g=2)

    # ---- transpose x on the tensor engine: xT[hh] is (h-part x t-free) ----
    xT = [cpool.tile([128, T], BF16, tag=f"xT{hh}", name=f"xT{hh}")
          for hh in range(4)]
    for hh in range(4):
        for tt in range(4):
            pt = tpp.tile([128, 128], F32, tag="tp", name=f"tp{hh}_{tt}")
            nc.tensor.matmul(pt[:], lhsT=xb_sb[:, tt, hh * 128:(hh + 1) * 128],
                             rhs=id_sb[:], start=True, stop=True)
            dst = xT[hh][:, tt * 128:(tt + 1) * 128]
            if (hh * 4 + tt) % 3 != 2:
                nc.vector.tensor_copy(dst, pt[:])
            else:
                nc.scalar.copy(dst, pt[:])

    # ---- U = B @ x^T  (S x T) ---------------------------------------------
    U_ps = spp.tile([128, T], F32, tag="U_ps")
    for hh in range(4):
        nc.tensor.matmul(U_ps[:], lhsT=Bt_sb[:, hh, :], rhs=xT[hh][:],
                         start=(hh == 0), stop=(hh == 3))
    U_sb = cpool.tile([128, T], BF16, tag="U_sb")
    nc.vector.tensor_copy(U_sb[:], U_ps[:])
    U_r = U_sb.rearrange("s (j r) -> s r j", r=Q)      # [128, Q, NCH]

    # ---- chunk summaries R ------------------------------------------------
    R_ps = spp.tile([128, NCH], F32, tag="scan_ps")
    for r in range(Q):
        nc.tensor.matmul(R_ps[:], lhsT=APOW_sb[:, r * S:(r + 1) * S],
                         rhs=U_r[:, r, :], start=(r == 0), stop=(r == Q - 1))
    R_sb = cpool.tile([128, NCH], BF16, tag="R_sb")
    nc.vector.tensor_copy(R_sb[:], R_ps[:])

    # ---- boundary states Z (block-Toeplitz matmuls over lags) -------------
    Z_ps = spp.tile([128, NCH], F32, tag="scan_ps")
    for L in range(LZ):
        nc.tensor.matmul(Z_ps[:, L + 1:NCH], lhsT=APQL_sb[:, L * S:(L + 1) * S],
                         rhs=R_sb[:, 0:NCH - 1 - L],
                         start=(L == 0), stop=(L == LZ - 1))
    Z_sb = cpool.tile([128, NCH], BF16, tag="Z_sb")
    nc.gpsimd.memset(Z_sb[:], 0.0)
    nc.vector.tensor_copy(Z_sb[:, 1:NCH], Z_ps[:, 1:NCH])

    # ---- c^T = Z^T G + triangular intra-chunk term (j-part x i-free) ------
    c_psT = spp.tile([NCH, Q], F32, tag="scan_ps")
    nc.tensor.matmul(c_psT[:], lhsT=Z_sb[:], rhs=GW_sb[:, 0:Q],
                     start=True, stop=False)
    for k in range(Q):
        nc.tensor.matmul(c_psT[:], lhsT=U_r[:, k, :],
                         rhs=GW_sb[:, Q + k * Q:Q + (k + 1) * Q],
                         start=False, stop=(k == Q - 1))
    c_sbT = cpool.tile([NCH, Q], F32, tag="c_sbT")
    nc.vector.tensor_copy(c_sbT[:], c_psT[:])

    # ---- reshape c^T (j x i) -> per-row column via a DRAM bounce ----------
    # c^T partition-major flat order IS t = j*Q + i; output rows are
    # t in [256, 512), i.e. the last 256 values.
    c_dram = dpool.tile([NCH, Q], F32, tag="c_dram")
    nc.sync.dma_start(c_dram[:], c_sbT[:])
    c_col = cpool.tile([128, 2], F32, tag="c_col")
    c_lin = c_dram.rearrange("j i -> (j i)")[TOUT:].rearrange(
        "(n p) -> p n", p=128)
    nc.sync.dma_start(c_col[:], c_lin)

    # ---- xD (after the scan chain so the c bounce starts ASAP) ------------
    y_pss = []
    for tt2 in range(2):
        y_ps = ypp.tile([128, H], F32, tag="y_ps", name=f"y_ps{tt2}")
        for hh in range(4):
            nc.tensor.matmul(
                y_ps[:],
                lhsT=xT[hh][:, 256 + tt2 * 128:256 + (tt2 + 1) * 128],
                rhs=Dt_sb[:, hh, :], start=(hh == 0), stop=(hh == 3))
        y_pss.append(y_ps)

    # ---- gelu + residual + stats (pass 1) ---------------------------------
    inv_h = 1.0 / H
    y_sbs, sums, sqs = [], [], []
    for tt2 in range(2):
        g_sb = wpool.tile([128, H], F32, tag="g_sb", name=f"g_sb{tt2}")
        nc.scalar.activation(g_sb[:], y_pss[tt2][:], AF.Gelu,
                             bias=c_col[:, tt2:tt2 + 1], scale=1.0)
        y_sb = wpool.tile([128, H], F32, tag=f"y_sb{tt2}", name=f"y_sb{tt2}")
        nc.vector.tensor_add(y_sb[:], g_sb[:], xres_sb[:, tt2, :])
        st6 = wpool.tile([128, 6], F32, tag="st6", name=f"st6_{tt2}")
        nc.vector.bn_stats(st6[:], y_sb[:])
        mv = wpool.tile([128, 2], F32, tag=f"mv{tt2}", name=f"mv{tt2}")
        nc.vector.bn_aggr(mv[:], st6[:])
        y_sbs.append(y_sb)
        sums.append(mv)
        sqs.append(None)

    # Dummy sqrt AFTER the squares: pulls the sqrt ACT-table load off the
    # critical tail (runs on ScalarE while the DVE works). +1e4 bias keeps
    # the argument positive.
    sq_scr = wpool.tile([128, 1], F32, tag="sq_scr")
    nc.scalar.activation(sq_scr[:], y_sbs[1][:, 0:1], AF.Sqrt,
                         bias=big_sb[:], scale=1.0)

    # ---- normalize and write out (pass 2) ---------------------------------
    for tt2 in range(2):
        y_sb, mv = y_sbs[tt2], sums[tt2]
        eng = nc.vector
        sd = wpool.tile([128, 1], F32, tag=f"sd{tt2}", name=f"sd{tt2}")
        nc.scalar.activation(sd[:], mv[:, 1:2], AF.Sqrt, bias=eps_sb[:], scale=1.0)
        iv = wpool.tile([128, 1], F32, tag=f"iv{tt2}", name=f"iv{tt2}")
        nc.vector.reciprocal(iv[:], sd[:])
        o_sb = wpool.tile([128, H], F32, tag="o_sb", name=f"o_sb{tt2}")
        eng.tensor_scalar(o_sb[:], y_sb[:], mv[:, 0:1], iv[:],
                          op0=ALU.subtract, op1=ALU.mult)
        if apply_gamma_beta:
            eng.tensor_tensor(o_sb[:], o_sb[:], gb_sb[:, 0, :], ALU.mult)
            eng.tensor_tensor(o_sb[:], o_sb[:], gb_sb[:, 1, :], ALU.add)
        nc.sync.dma_start(yout[tt2 * 128:(tt2 + 1) * 128, :], o_sb[:])

    ctx.close()


def _build_program(apply_gamma_beta, LZ):
    nc = bacc.Bacc("TRN2", target_bir_lowering=False, debug=False,
                   enable_asserts=False, num_devices=NCORES)
    fbt = _O_AQ + LZ * S + Q + Q * Q
    f32tot = 4 * H if apply_gamma_beta else 2 * H
    aps = {
        "xb": nc.dram_tensor("xb", (T, H), BF16, kind="ExternalInput").ap(),
        "pbt": nc.dram_tensor("pbt", (128, fbt), BF16,
                              kind="ExternalInput").ap(),
        "pdt": nc.dram_tensor("pdt", (128, 4 * H), BF16,
                              kind="ExternalInput").ap(),
        "p32": nc.dram_tensor("p32", (128, f32tot), F32,
                              kind="ExternalInput").ap(),
        "yout": nc.dram_tensor("yout", (TOUT, H), F32, kind="ExternalOutput").ap(),
    }
    with tile.TileContext(nc) as tc:
        _emit(tc, aps, apply_gamma_beta, LZ)
    nc.compile()
    return nc


def _prepare_in_maps(x, A, Bm, Cm, D, gamma, beta, apply_gamma_beta):
    APOW, APQL, GW, LZ = _host_weights(A, Bm, Cm)

    def part_major(m, inner):
        # (4*128, inner) -> (128, 4*inner):  row (hh*128+p) -> [p, hh*inner:]
        return np.ascontiguousarray(
            m.reshape(4, 128, inner).transpose(1, 0, 2).reshape(128, 4 * inner))

    pbt = np.concatenate([
        part_major(Bm.T, S),
        APOW,
        APQL,
        GW,
    ], axis=1).astype(BF16_NP)
    pdt = np.ascontiguousarray(part_major(D.T, H)).astype(BF16_NP)

    in_maps = []
    for core in range(NCORES):
        b, half = core // 2, core % 2
        if half == 0:
            xb = np.concatenate(
                [np.zeros((TOUT, H), np.float32), x[b, :TOUT]], axis=0)
        else:
            xb = x[b]
        xres = xb[TOUT:].reshape(2, 128, H).transpose(1, 0, 2).reshape(128, 2 * H)
        p32 = [xres]
        if apply_gamma_beta:
            p32.append(np.broadcast_to(gamma, (128, H)))
            p32.append(np.broadcast_to(beta, (128, H)))
        in_maps.append({
            "xb": np.ascontiguousarray(xb).astype(BF16_NP),
            "pbt": pbt,
            "pdt": pdt,
            "p32": np.ascontiguousarray(
                np.concatenate(p32, axis=1).astype(np.float32)),
        })
    return in_maps, LZ


def _run(inputs, trace=False):
    x = np.asarray(inputs["x"], np.float32)
    A = np.asarray(inputs["A"], np.float32)
    Bm = np.asarray(inputs["B"], np.float32)
    Cm = np.asarray(inputs["C"], np.float32)
    D = np.asarray(inputs["D"], np.float32)
    gamma = np.asarray(inputs["gamma"], np.float32)
    beta = np.asarray(inputs["beta"], np.float32)

    apply_gamma_beta = not (np.all(gamma == 1.0) and np.all(beta == 0.0))
    in_maps, LZ = _prepare_in_maps(x, A, Bm, Cm, D, gamma, beta,
                                   apply_gamma_beta)
    nc = _build_program(apply_gamma_beta, LZ)
    res = bass_utils.run_bass_kernel_spmd(
        nc, in_maps, core_ids=list(range(NCORES)), trace=trace)
    y = np.empty((BSZ, T, H), np.float32)
    for core in range(NCORES):
        b, half = core // 2, core % 2
        y[b, half * TOUT:(half + 1) * TOUT, :] = res.results[core]["yout"]
    return y, res


def kernel(**inputs):
    y, _ = _run(inputs, trace=False)
    return y


def kernel_traced(**inputs):
    return _run(inputs, trace=True)


# revision 23
# speedup vs baseline: 1.3959x; 1.3959x over previous
"""Trainium2 Bass kernel for nn_SSMLayer_17514876633683.

Math: the reference SSM state update broadcasts the input over H and starts
from zero state, so state[b,:,h] is identical for every h.  The whole layer
collapses to:
    z_t[b]    = A @ z_{t-1}[b] + B @ x[b,t]          (z in R^S, S=128)
    c[b,t]    = Cbar . z_t[b]                         (Cbar = C.mean(0))
    y_pre     = c[b,t] + (x @ D.T)[b,t,:]
    y         = LN(gelu(y_pre) + x) * gamma + beta

Sharding: 8 cores = 4 batches x 2 time-halves.  Every core runs the same
SPMD program: "scan all 512 steps of the provided x, output rows 256..511".
The first-half core of each batch receives x zero-padded at the front so its
output rows land in [256, 512) too.

Scan mapping on device (per core, its batch):
  U = B @ x^T                               (S x T)       - PE matmuls
  R_j = sum_r A^(Q-1-r) U[:, jQ+r]          (chunk summaries, Q=16, 32 chunks)
  Z_j = sum_{L<LZ} (A^Q)^L R_{j-1-L}        (chunk-boundary states; LZ lag
                                             matmuls; higher lags dropped when
                                             ||(A^Q)^L|| is negligible)
  c^T[j,i] = g_i . Z_j + sum_{k<i} g_{i-1-k} . U[:, jQ+k]  (g_k = (A^T)^k Cbar)
All A-power / g weight matrices are precomputed host-side from the inputs.
Matmul operands are bf16 (fp32 PSUM accumulation); the residual/layernorm
path stays fp32.  x is transposed on the (pre-warmed) tensor engine; all
weights arrive in one packed DMA per dtype.
"""

import sys
from contextlib import ExitStack

sys.path.insert(0, "/opt/trn_rl_repo")

import ml_dtypes
import numpy as np

import concourse.bass as bass  # noqa: F401
import concourse.mybir as mybir
import concourse.tile as tile
from concourse import bacc, bass_utils
from concourse.masks import make_identity

# Problem shapes (hardcoded per the harness contract).
BSZ, T, H, S = 4, 512, 512, 128
Q = 16           # scan chunk length
NCH = T // Q     # 32 chunks
TOUT = 256       # output rows per core
LN_EPS = 1e-5
NCORES = 8
NWARM = 10       # upfront PE warmup matmuls (more interleaved as fillers)
TRUNC_TOL = 1e-5

F32 = mybir.dt.float32
BF16 = mybir.dt.bfloat16
BF16_NP = ml_dtypes.bfloat16
AF = mybir.ActivationFunctionType
ALU = mybir.AluOpType

# PBT element offsets (bf16 pack: B^T | APOW | APQL | GW | c-masks)
_O_BT = 0
_O_AP = _O_BT + 4 * S
_O_AQ = _O_AP + Q * S
# _O_MS(LZ) = _O_AQ + LZ*S + Q + Q*Q  (two (32x128) c-scatter masks)


def _host_weights(A, Bm, Cm):
    """Precompute scan weights; returns (APOW, APQL, GW, LZ) as float64."""
    A64 = A.astype(np.float64)
    Cbar = Cm.astype(np.float64).mean(axis=0)          # (S,)

    pows = [np.eye(S)]
    for _ in range(Q):
        pows.append(pows[-1] @ A64)                    # pows[k] = A^k
    A16 = pows[Q]

    # lhsT tiles for R: column block r holds (A^(Q-1-r))^T
    APOW = np.concatenate([pows[Q - 1 - r].T for r in range(Q)], axis=1)

    # boundary-lag powers, truncated once ||(A^Q)^L|| is negligible
    q16 = [np.eye(S)]
    while len(q16) < NCH - 1:
        nxt = q16[-1] @ A16
        if np.linalg.norm(nxt, 2) < TRUNC_TOL:
            break
        q16.append(nxt)
    LZ = len(q16)
    APQL = np.concatenate([m.T for m in q16], axis=1)

    g = [pows[k].T @ Cbar for k in range(Q)]           # g_k = (A^T)^k Cbar
    G16 = np.stack(g, axis=1)                          # (S, Q)
    WTRI = np.zeros((S, Q * Q))
    for k in range(Q):
        for i in range(Q):
            if i > k:
                WTRI[:, k * Q + i] = g[i - 1 - k]
    GW = np.concatenate([G16, WTRI], axis=1)           # (S, Q + Q*Q)

    return APOW, APQL, GW, LZ


def _emit(tc, aps, apply_gamma_beta, LZ):
    nc = tc.nc
    xb, pbt, pdt, p32, yout = (aps["xb"], aps["pbt"], aps["pdt"], aps["p32"],
                               aps["yout"])

    ctx = ExitStack()
    cpool = ctx.enter_context(tc.tile_pool(name="const", bufs=1))
    wpool = ctx.enter_context(tc.tile_pool(name="work", bufs=2))
    tpp = ctx.enter_context(tc.tile_pool(name="tpp", bufs=3, space="PSUM"))
    spp = ctx.enter_context(tc.tile_pool(name="spp", bufs=1, space="PSUM"))
    ypp = ctx.enter_context(tc.tile_pool(name="ypp", bufs=2, space="PSUM"))

    # ---- PE warmup + gelu table preload (runs while input DMAs land) ------
    # One accumulation group so the matmuls pipeline back-to-back and trip
    # the HAM un-throttle (isolated matmuls never warm the clock gate).
    warm_sb = cpool.tile([128, 256], BF16, tag="warm_sb")
    nc.gpsimd.memset(warm_sb[:], 0.0)
    wp = tpp.tile([128, 256], F32, tag="tp", name="warm_ps")
    for i in range(NWARM):
        nc.tensor.matmul(wp[:], lhsT=warm_sb[:, :128], rhs=warm_sb[:],
                         start=(i == 0), stop=(i == NWARM - 1))
    gsc = cpool.tile([128, 1], F32, tag="gsc")
    nc.gpsimd.memset(gsc[:], 0.0)
    nc.scalar.activation(gsc[:], gsc[:], AF.Gelu)

    eps_sb = cpool.tile([128, 1], F32, tag="eps_sb")
    nc.gpsimd.memset(eps_sb[:], LN_EPS)
    big_sb = cpool.tile([128, 1], F32, tag="big_sb")
    nc.gpsimd.memset(big_sb[:], 1.0e4)
    zer_sb = cpool.tile([128, 1], F32, tag="zer_sb")
    nc.gpsimd.memset(zer_sb[:], 0.0)
    ones32 = cpool.tile([32, H], BF16, tag="ones32")
    nc.gpsimd.memset(ones32[:], 1.0)

    # ---- identity built on device (no DMA on the transpose path) ----------
    id_sb = cpool.tile([128, 128], BF16, tag="id_sb")
    make_identity(nc, id_sb[:])

    # ---- input loads: ordered by need-time --------------------------------
    xb_sb = cpool.tile([128, 4, H], BF16, tag="xb_sb")
    nc.sync.dma_start(xb_sb[:], xb.rearrange("(tt p) h -> p tt h", p=128))
    PBT = cpool.tile([128, pbt.shape[1]], BF16, tag="PBT")
    nc.sync.dma_start(PBT[:], pbt)
    PDT = cpool.tile([128, 4, H], BF16, tag="PDT")
    nc.sync.dma_start(PDT[:], pdt.rearrange("p (hh o) -> p hh o", hh=4))
    P32 = cpool.tile([128, p32.shape[1]], F32, tag="P32")
    nc.gpsimd.dma_start(P32[:], p32)

    Bt_sb = PBT[:, _O_BT:_O_BT + 4 * S].rearrange("p (hh s) -> p hh s", hh=4)
    Dt_sb = PDT
    APOW_sb = PBT[:, _O_AP:_O_AP + Q * S]
    APQL_sb = PBT[:, _O_AQ:_O_AQ + LZ * S]
    GW_sb = PBT[:, _O_AQ + LZ * S:_O_AQ + LZ * S + Q + Q * Q]
    xres_sb = P32[:, 0:2 * H].rearrange("p (tt h) -> p tt h", tt=2)
    if apply_gamma_beta:
        gb_sb = P32[:, 2 * H:4 * H].rearrange("p (g h) -> p g h", g=2)

    # ---- transpose x on the tensor engine: xT[hh] is (h-part x t-free) ----
    xT = [cpool.tile([128, T], BF16, tag=f"xT{hh}", name=f"xT{hh}")
          for hh in range(4)]
    for hh in range(4):
        pt = tpp.tile([128, T], F32, tag="tp", name=f"tp{hh}")
        for tt in range(4):
            nc.tensor.matmul(pt[:, tt * 128:(tt + 1) * 128],
                             lhsT=xb_sb[:, tt, hh * 128:(hh + 1) * 128],
                             rhs=id_sb[:], start=True, stop=True)
            # dep-free filler matmul keeps the PE duty cycle high so the
            # HAM clock gate stays open through the copy-bound stretch
            nc.tensor.matmul(wp[:], lhsT=warm_sb[:, :128], rhs=warm_sb[:],
                             start=True, stop=True)
        if hh % 2 == 0:
            nc.vector.tensor_copy(xT[hh][:], pt[:])
        else:
            nc.scalar.copy(xT[hh][:], pt[:])

    # ---- U = B @ x^T  (S x T) ---------------------------------------------
    U_ps = spp.tile([128, T], F32, tag="U_ps")
    for hh in range(4):
        nc.tensor.matmul(U_ps[:], lhsT=Bt_sb[:, hh, :], rhs=xT[hh][:],
                         start=(hh == 0), stop=(hh == 3))
    U_sb = cpool.tile([128, T], BF16, tag="U_sb")
    nc.vector.tensor_copy(U_sb[:], U_ps[:])
    U_r = U_sb.rearrange("s (j r) -> s r j", r=Q)      # [128, Q, NCH]

    # ---- chunk summaries R ------------------------------------------------
    R_ps = spp.tile([128, NCH], F32, tag="scan_ps")
    for r in range(Q):
        nc.tensor.matmul(R_ps[:], lhsT=APOW_sb[:, r * S:(r + 1) * S],
                         rhs=U_r[:, r, :], start=(r == 0), stop=(r == Q - 1))
    R_sb = cpool.tile([128, NCH], BF16, tag="R_sb")
    nc.vector.tensor_copy(R_sb[:], R_ps[:])

    # ---- boundary states Z (block-Toeplitz matmuls over lags) -------------
    Z_ps = spp.tile([128, NCH], F32, tag="scan_ps")
    for L in range(LZ):
        nc.tensor.matmul(Z_ps[:, L + 1:NCH], lhsT=APQL_sb[:, L * S:(L + 1) * S],
                         rhs=R_sb[:, 0:NCH - 1 - L],
                         start=(L == 0), stop=(L == LZ - 1))
    Z_sb = cpool.tile([128, NCH], BF16, tag="Z_sb")
    nc.gpsimd.memset(Z_sb[:], 0.0)
    nc.vector.tensor_copy(Z_sb[:, 1:NCH], Z_ps[:, 1:NCH])

    # ---- c^T = Z^T G + triangular intra-chunk term (j-part x i-free) ------
    c_psT = spp.tile([NCH, Q], F32, tag="scan_ps")
    nc.tensor.matmul(c_psT[:], lhsT=Z_sb[:], rhs=GW_sb[:, 0:Q],
                     start=True, stop=False)
    for k in range(Q):
        nc.tensor.matmul(c_psT[:], lhsT=U_r[:, k, :],
                         rhs=GW_sb[:, Q + k * Q:Q + (k + 1) * Q],
                         start=False, stop=(k == Q - 1))
    # ---- scatter c into per-row lhsT columns: lhsTc_n[j,p] ----------------
    # lhsTc_n[j, p] = c^T[j, p%16] * [j == 16 + 8n + p//16]; then
    # y_ps[p, h] += sum_j lhsTc_n[j, p] * 1  adds c[t(p)] to every h.
    o_ms = _O_AQ + LZ * S + Q + Q * Q
    c_bc = c_psT[:, None, :].to_broadcast((NCH, 8, Q))
    lhsTcs = []
    for n in range(2):
        msk = pbt_msk = PBT[0:NCH, o_ms + n * 128:o_ms + (n + 1) * 128]
        lhsTc = cpool.tile([NCH, 128], BF16, tag=f"lhsTc{n}", name=f"lhsTc{n}")
        nc.vector.tensor_tensor(
            lhsTc.rearrange("j (jm i) -> j jm i", jm=8), c_bc,
            msk.rearrange("j (jm i) -> j jm i", jm=8), ALU.mult)
        lhsTcs.append(lhsTc)

    # ---- xD + c-injection -------------------------------------------------
    y_pss = []
    for tt2 in range(2):
        y_ps = ypp.tile([128, H], F32, tag="y_ps", name=f"y_ps{tt2}")
        for hh in range(4):
            nc.tensor.matmul(
                y_ps[:],
                lhsT=xT[hh][:, 256 + tt2 * 128:256 + (tt2 + 1) * 128],
                rhs=Dt_sb[:, hh, :], start=(hh == 0), stop=False)
        nc.tensor.matmul(y_ps[:], lhsT=lhsTcs[tt2][:], rhs=ones32[:],
                         start=False, stop=True)
        y_pss.append(y_ps)

    # ---- gelu + residual + stats (pass 1) ---------------------------------
    inv_h = 1.0 / H
    y_sbs, sums, sqs = [], [], []
    for tt2 in range(2):
        g_sb = wpool.tile([128, H], F32, tag="g_sb", name=f"g_sb{tt2}")
        nc.scalar.activation(g_sb[:], y_pss[tt2][:], AF.Gelu,
                             bias=zer_sb[:], scale=1.0)
        y_sb = wpool.tile([128, H], F32, tag=f"y_sb{tt2}", name=f"y_sb{tt2}")
        nc.vector.tensor_add(y_sb[:], g_sb[:], xres_sb[:, tt2, :])
        st6 = wpool.tile([128, 6], F32, tag="st6", name=f"st6_{tt2}")
        nc.vector.bn_stats(st6[:], y_sb[:])
        mv = wpool.tile([128, 2], F32, tag=f"mv{tt2}", name=f"mv{tt2}")
        nc.vector.bn_aggr(mv[:], st6[:])
        y_sbs.append(y_sb)
        sums.append(mv)
        sqs.append(None)

    # Dummy sqrt AFTER the squares: pulls the sqrt ACT-table load off the
    # critical tail (runs on ScalarE while the DVE works). +1e4 bias keeps
    # the argument positive.
    sq_scr = wpool.tile([128, 1], F32, tag="sq_scr")
    nc.scalar.activation(sq_scr[:], y_sbs[1][:, 0:1], AF.Sqrt,
                         bias=big_sb[:], scale=1.0)

    # ---- normalize and write out (pass 2) ---------------------------------
    for tt2 in range(2):
        y_sb, mv = y_sbs[tt2], sums[tt2]
        eng = nc.vector
        sd = wpool.tile([128, 1], F32, tag=f"sd{tt2}", name=f"sd{tt2}")
        nc.scalar.activation(sd[:], mv[:, 1:2], AF.Sqrt, bias=eps_sb[:], scale=1.0)
        iv = wpool.tile([128, 1], F32, tag=f"iv{tt2}", name=f"iv{tt2}")
        nc.vector.reciprocal(iv[:], sd[:])
        o_sb = wpool.tile([128, H], F32, tag="o_sb", name=f"o_sb{tt2}")
        eng.tensor_scalar(o_sb[:], y_sb[:], mv[:, 0:1], iv[:],
                          op0=ALU.subtract, op1=ALU.mult)
        if apply_gamma_beta:
            eng.tensor_tensor(o_sb[:], o_sb[:], gb_sb[:, 0, :], ALU.mult)
            eng.tensor_tensor(o_sb[:], o_sb[:], gb_sb[:, 1, :], ALU.add)
        nc.sync.dma_start(yout[tt2 * 128:(tt2 + 1) * 128, :], o_sb[:])

    ctx.close()


def _build_program(apply_gamma_beta, LZ):
    nc = bacc.Bacc("TRN2", target_bir_lowering=False, debug=False,
                   enable_asserts=False, num_devices=NCORES)
    fbt = _O_AQ + LZ * S + Q + Q * Q + 256
    f32tot = 4 * H if apply_gamma_beta else 2 * H
    aps = {
        "xb": nc.dram_tensor("xb", (T, H), BF16, kind="ExternalInput").ap(),
        "pbt": nc.dram_tensor("pbt", (128, fbt), BF16,
                              kind="ExternalInput").ap(),
        "pdt": nc.dram_tensor("pdt", (128, 4 * H), BF16,
                              kind="ExternalInput").ap(),
        "p32": nc.dram_tensor("p32", (128, f32tot), F32,
                              kind="ExternalInput").ap(),
        "yout": nc.dram_tensor("yout", (TOUT, H), F32, kind="ExternalOutput").ap(),
    }
    with tile.TileContext(nc) as tc:
        _emit(tc, aps, apply_gamma_beta, LZ)
    nc.compile()
    return nc


def _prepare_in_maps(x, A, Bm, Cm, D, gamma, beta, apply_gamma_beta):
    APOW, APQL, GW, LZ = _host_weights(A, Bm, Cm)

    def part_major(m, inner):
        # (4*128, inner) -> (128, 4*inner):  row (hh*128+p) -> [p, hh*inner:]
        return np.ascontiguousarray(
            m.reshape(4, 128, inner).transpose(1, 0, 2).reshape(128, 4 * inner))

    msk = np.zeros((128, 256))
    for n in range(2):
        for p in range(128):
            msk[16 + 8 * n + p // 16, n * 128 + p] = 1.0
    pbt = np.concatenate([
        part_major(Bm.T, S),
        APOW,
        APQL,
        GW,
        msk,
    ], axis=1).astype(BF16_NP)
    pdt = np.ascontiguousarray(part_major(D.T, H)).astype(BF16_NP)

    in_maps = []
    for core in range(NCORES):
        b, half = core // 2, core % 2
        if half == 0:
            xb = np.concatenate(
                [np.zeros((TOUT, H), np.float32), x[b, :TOUT]], axis=0)
        else:
            xb = x[b]
        xres = xb[TOUT:].reshape(2, 128, H).transpose(1, 0, 2).reshape(128, 2 * H)
        p32 = [xres]
        if apply_gamma_beta:
            p32.append(np.broadcast_to(gamma, (128, H)))
            p32.append(np.broadcast_to(beta, (128, H)))
        in_maps.append({
            "xb": np.ascontiguousarray(xb).astype(BF16_NP),
            "pbt": pbt,
            "pdt": pdt,
            "p32": np.ascontiguousarray(
                np.concatenate(p32, axis=1).astype(np.float32)),
        })
    return in_maps, LZ


def _run(inputs, trace=False):
    x = np.asarray(inputs["x"], np.float32)
    A = np.asarray(inputs["A"], np.float32)
    Bm = np.asarray(inputs["B"], np.float32)
    Cm = np.asarray(inputs["C"], np.float32)
    D = np.asarray(inputs["D"], np.float32)
    gamma = np.asarray(inputs["gamma"], np.float32)
    beta = np.asarray(inputs["beta"], np.float32)

    apply_gamma_beta = not (np.all(gamma == 1.0) and np.all(beta == 0.0))
    in_maps, LZ = _prepare_in_maps(x, A, Bm, Cm, D, gamma, beta,
                                   apply_gamma_beta)
    nc = _build_program(apply_gamma_beta, LZ)
    res = bass_utils.run_bass_kernel_spmd(
        nc, in_maps, core_ids=list(range(NCORES)), trace=trace)
    y = np.empty((BSZ, T, H), np.float32)
    for core in range(NCORES):
        b, half = core // 2, core % 2
        y[b, half * TOUT:(half + 1) * TOUT, :] = res.results[core]["yout"]
    return y, res


def kernel(**inputs):
    y, _ = _run(inputs, trace=False)
    return y


def kernel_traced(**inputs):
    return _run(inputs, trace=True)
